# revision 7
# baseline (speedup 1.0000x reference)
"""MoE update-MLP Trainium2 kernel (8-core SPMD, sparse top-2 routing).

Problem: x (4,192,128,128); a per-pixel router picks top-2 of 8 experts; each
expert is a 3-layer 1x1-conv MLP (192->384 gelu ->384 gelu ->192); output is
the gate-weighted sum over experts.

Strategy (vs. the dense baseline that ran every expert on every pixel):
exploit the top-2 sparsity. The router is a tiny 8x8 per-pixel linear --
0.5% of the FLOPs -- so it is evaluated on the host, which then groups the
131072 (pixel, expert) pairs by expert, pads each expert's run to a multiple
of 512, and splits the resulting tile list evenly across the 8 cores
(perfect load balance even though the router is expert-skewed). Each 512-pair
tile is single-expert; the host streams that tile's weights (bf16, one
combined slab DMA per tile) alongside the gathered x columns, so the device
program is completely uniform and SPMD-identical -- all per-core/per-tile
variation lives in the input data.

Per 512-pair tile on each core (all matmuls bf16, PSUM fp32):
 - L1: K=192 split 96+96 (no zero pad), M=384 -> 6 matmuls, exact Gelu+bias
   on ACT -> h1 (bf16)
 - L2: K=384 (3), M=384 (3) -> 9 matmuls, Gelu+bias -> h2 (bf16)
 - L3: K=384 (3), M=192 (128+64) -> 6 matmuls; DVE copies PSUM into one
   bf16 output tile; single DMA out
The b3 bias, the gate weighting and the scatter back to pixel order all
happen on the host: out[pix] = g1*(y[q1]+b3[e1]) + g2*(y[q2]+b3[e2]); each
pixel has exactly one rank-1 and one rank-2 pair, so the combine is two
fancy-indexed gathers.

21 matmuls x 512 cols per tile, 33 tiles/core = ~1/4 of the dense baseline's
PE work.
"""

import numpy as np
from ml_dtypes import bfloat16

import concourse.bacc as bacc
import concourse.mybir as mybir
import concourse.tile as tile
from concourse.bass_utils import run_bass_kernel_spmd

F32 = mybir.dt.float32
BF16 = mybir.dt.bfloat16
AF = mybir.ActivationFunctionType
ALU = mybir.AluOpType

N_CORES = 8
B, IN_C, H, W = 4, 192, 128, 128
R_C, E, HID, OUT_C = 8, 8, 384, 192
P = B * H * W                 # 65536 pixels
TILE = 512                    # pixel-expert pairs per compute tile
NT = 33                       # tiles per core
NT_TOT = NT * N_CORES         # 264 tile capacity (needs >= ~257 on average)
KH = 96                       # L1 contraction split: 192 = 96 + 96

# weight-slab column layout (bf16): w1 | w2 | w3 | b1 | b2
_W1C = 0          # [:96, 384*k + 128*m] for k in 0..1
_W2C = 768        # [:,  768 + 384*k + 128*m] for k in 0..2
_W3C = 1920       # [:,  1920 + 192*k + 128*m] for k in 0..2
_B1C = 2496       # [:, 2496+m]
_B2C = 2499       # [:, 2499+m]
WC = 2504         # padded total columns

_nc_cache: dict = {}


def _build(compile: bool = True):
    """Build the (SPMD-identical) Bass program for one core."""
    nc = bacc.Bacc("TRN2", target_bir_lowering=False, debug=False)

    xg_in = nc.declare_dram_parameter("xg", [NT, KH, 2, TILE], BF16, isOutput=False)
    ws_in = nc.declare_dram_parameter("ws", [NT, 128, WC], BF16, isOutput=False)
    y_out = nc.declare_dram_parameter("y", [NT, 128, 2, TILE], BF16, isOutput=True)

    with tile.TileContext(nc) as tc:
        with (
            tc.tile_pool(name="xpool", bufs=3) as xpool,
            tc.tile_pool(name="wpool", bufs=3) as wpool,
            tc.tile_pool(name="hpool", bufs=6) as hpool,
            tc.tile_pool(name="ypool", bufs=3) as ypool,
            tc.tile_pool(name="psL1", bufs=2, space="PSUM") as psL1,
            tc.tile_pool(name="psL2", bufs=3, space="PSUM") as psL2,
            tc.tile_pool(name="psL3", bufs=3, space="PSUM") as psL3,
        ):
            for t in range(NT):
                x_sb = xpool.tile([KH, 2, TILE], BF16, tag="x")
                nc.sync.dma_start(x_sb[:], xg_in[t])
                w_sb = wpool.tile([128, WC], BF16, tag="w")
                nc.gpsimd.dma_start(w_sb[:], ws_in[t])

                h1 = []
                for m in range(3):
                    ps1 = psL1.tile([128, TILE], F32, tag="ps1")
                    for k in range(2):
                        c0 = _W1C + 384 * k + 128 * m
                        nc.tensor.matmul(
                            ps1[:],
                            w_sb[:KH, c0 : c0 + 128],
                            x_sb[:, k, :],
                            start=(k == 0),
                            stop=(k == 1),
                        )
                    h1_m = hpool.tile([128, TILE], BF16, tag="h1")
                    nc.scalar.activation(
                        h1_m[:], ps1[:], AF.Gelu,
                        bias=w_sb[:, _B1C + m : _B1C + m + 1],
                    )
                    h1.append(h1_m)

                h2 = []
                for m in range(3):
                    ps2 = psL2.tile([128, TILE], F32, tag="ps2")
                    for k in range(3):
                        c0 = _W2C + 384 * k + 128 * m
                        nc.tensor.matmul(
                            ps2[:],
                            w_sb[:, c0 : c0 + 128],
                            h1[k][:],
                            start=(k == 0),
                            stop=(k == 2),
                        )
                    h2_m = hpool.tile([128, TILE], BF16, tag="h2")
                    nc.scalar.activation(
                        h2_m[:], ps2[:], AF.Gelu,
                        bias=w_sb[:, _B2C + m : _B2C + m + 1],
                    )
                    h2.append(h2_m)

                y_sb = ypool.tile([128, 2, TILE], BF16, tag="y")
                for m, rows in ((0, 128), (1, OUT_C - 128)):
                    ps3 = psL3.tile([128, TILE], F32, tag="ps3")
                    for k in range(3):
                        c0 = _W3C + 192 * k + 128 * m
                        nc.tensor.matmul(
                            ps3[:rows],
                            w_sb[:, c0 : c0 + rows],
                            h2[k][:],
                            start=(k == 0),
                            stop=(k == 2),
                        )
                    nc.vector.tensor_copy(y_sb[:rows, m, :], ps3[:rows])
                nc.sync.dma_start(y_out[t], y_sb[:])

    if compile:
        nc.compile()
    return nc


def _get_nc():
    if "nc" not in _nc_cache:
        _nc_cache["nc"] = _build()
    return _nc_cache["nc"]


def _route(router_input, router_W, router_b):
    """Host-side router: per-pixel top-2 experts and gates.

    Matches the reference: logits -> top-2 -> softmax over the two kept
    logits (2-way softmax == sigmoid of the difference).
    """
    f = np.float32
    r = np.asarray(router_input, f).reshape(B, R_C, H * W)
    Wr = np.asarray(router_W, f)
    br = np.asarray(router_b, f)
    lt = np.einsum("ec,bcp->bpe", Wr, r).reshape(P, E) + br[None, :]
    ord2 = np.argsort(-lt, axis=1, kind="stable")[:, :2]
    t1 = ord2[:, 0].astype(np.int32)
    t2 = ord2[:, 1].astype(np.int32)
    l12 = np.take_along_axis(lt, ord2, axis=1)
    g1 = 1.0 / (1.0 + np.exp(l12[:, 1] - l12[:, 0]))
    g2 = (1.0 - g1).astype(f)
    return t1, t2, g1.astype(f), g2


def _plan(t1, t2):
    """Group pairs by expert into 512-pair tiles; assign tiles to cores.

    Returns (cols, texp, q1, q2): cols[j] = source pixel of global pair
    column j (-1 = padding); texp[t] = expert of global tile t (-1 = empty);
    q1[p]/q2[p] = global pair column holding pixel p's rank-1/rank-2
    contribution (-1 if dropped on overflow -- cannot happen for this
    problem's 131072+padding <= 264*512, but handled for safety).
    """
    pixA = [np.flatnonzero(t1 == e) for e in range(E)]
    pixB = [np.flatnonzero(t2 == e) for e in range(E)]
    counts = np.array([len(pixA[e]) + len(pixB[e]) for e in range(E)])
    tiles_e = np.maximum(1, (counts + TILE - 1) // TILE)
    # overflow safety: drop pairs (rank-2 first) from the fullest experts
    while tiles_e.sum() > NT_TOT:
        e = int(np.argmax(counts - (tiles_e - 1) * TILE))
        drop = int(counts[e] - (tiles_e[e] - 1) * TILE)
        nB = len(pixB[e])
        db = min(drop, nB)
        pixB[e] = pixB[e][: nB - db]
        if db < drop:
            pixA[e] = pixA[e][: len(pixA[e]) - (drop - db)]
        counts[e] -= drop
        tiles_e[e] -= 1

    cols = np.full(NT_TOT * TILE, -1, np.int64)
    texp = np.full(NT_TOT, -1, np.int64)
    q1 = np.full(P, -1, np.int64)
    q2 = np.full(P, -1, np.int64)
    off = 0
    for e in range(E):
        nA, nB = len(pixA[e]), len(pixB[e])
        cols[off : off + nA] = pixA[e]
        cols[off + nA : off + nA + nB] = pixB[e]
        q1[pixA[e]] = off + np.arange(nA)
        q2[pixB[e]] = off + nA + np.arange(nB)
        t0 = off // TILE
        texp[t0 : t0 + tiles_e[e]] = e
        off += int(tiles_e[e]) * TILE
    return cols, texp, q1, q2


def _weight_slabs(W1, b1, W2, b2, W3):
    """Per-expert [128, WC] bf16 slabs in the device layout."""
    f = np.float32
    sl = np.zeros((E, 128, WC), f)
    w1t = np.transpose(np.asarray(W1, f), (0, 2, 1))  # [E, 192, HID]
    sl[:, :KH, _W1C : _W1C + 384] = w1t[:, :KH, :]
    sl[:, :KH, _W1C + 384 : _W1C + 768] = w1t[:, KH:, :]
    w2t = np.transpose(np.asarray(W2, f), (0, 2, 1))  # [E, 384, HID]
    for k in range(3):
        sl[:, :, _W2C + 384 * k : _W2C + 384 * (k + 1)] = w2t[
            :, 128 * k : 128 * (k + 1), :
        ]
    w3t = np.transpose(np.asarray(W3, f), (0, 2, 1))  # [E, 384, OUT_C]
    for k in range(3):
        sl[:, :, _W3C + 192 * k : _W3C + 192 * (k + 1)] = w3t[
            :, 128 * k : 128 * (k + 1), :
        ]
    b1a = np.asarray(b1, f).reshape(E, 3, 128)
    b2a = np.asarray(b2, f).reshape(E, 3, 128)
    for m in range(3):
        sl[:, :, _B1C + m] = b1a[:, m, :]
        sl[:, :, _B2C + m] = b2a[:, m, :]
    return sl.astype(bfloat16)


def make_in_maps(x, W1, b1, W2, b2, W3, cols, texp):
    f = np.float32
    wsl = _weight_slabs(W1, b1, W2, b2, W3)
    x16 = (
        np.asarray(x, f).transpose(1, 0, 2, 3).reshape(IN_C, P).astype(bfloat16)
    )  # [192, P]

    in_maps = []
    for c in range(N_CORES):
        colc = cols[c * NT * TILE : (c + 1) * NT * TILE]
        xc = np.zeros((IN_C, NT * TILE), bfloat16)
        valid = colc >= 0
        xc[:, valid] = x16[:, colc[valid]]
        xr = xc.reshape(IN_C, NT, TILE)
        xg = np.empty((NT, KH, 2, TILE), bfloat16)
        xg[:, :, 0, :] = xr[0:KH].transpose(1, 0, 2)
        xg[:, :, 1, :] = xr[KH : 2 * KH].transpose(1, 0, 2)
        tc_ = texp[c * NT : (c + 1) * NT]
        tcw = np.where(tc_ >= 0, tc_, 0)
        in_maps.append({"xg": xg, "ws": np.ascontiguousarray(wsl[tcw])})
    return in_maps


def kernel(x, router_input, router_W, router_b, W1, b1, W2, b2, W3, b3, **run_kwargs):
    nc = _get_nc()
    t1, t2, g1, g2 = _route(router_input, router_W, router_b)
    cols, texp, q1, q2 = _plan(t1, t2)
    in_maps = make_in_maps(x, W1, b1, W2, b2, W3, cols, texp)
    res = run_bass_kernel_spmd(nc, in_maps, list(range(N_CORES)), **run_kwargs)
    Y = np.concatenate(
        [np.asarray(res.results[c]["y"]) for c in range(N_CORES)], axis=0
    )  # [NT_TOT, 128, 2, TILE] bf16
    Yg = np.empty((OUT_C, NT_TOT * TILE), np.float32)
    Yg[0:128] = Y[:, :, 0, :].transpose(1, 0, 2).reshape(128, -1)
    Yg[128:OUT_C] = (
        Y[:, : OUT_C - 128, 1, :].transpose(1, 0, 2).reshape(OUT_C - 128, -1)
    )
    b3f = np.asarray(b3, np.float32)  # [E, OUT_C]
    if (q1 >= 0).all() and (q2 >= 0).all():
        outf = (Yg[:, q1] + b3f[t1].T) * g1[None, :] + (
            Yg[:, q2] + b3f[t2].T
        ) * g2[None, :]
    else:
        outf = np.zeros((OUT_C, P), np.float32)
        m1 = q1 >= 0
        m2 = q2 >= 0
        outf[:, m1] = (Yg[:, q1[m1]] + b3f[t1[m1]].T) * g1[m1][None, :]
        outf[:, m2] += (Yg[:, q2[m2]] + b3f[t2[m2]].T) * g2[m2][None, :]
    out = np.ascontiguousarray(
        outf.reshape(OUT_C, B, H, W).transpose(1, 0, 2, 3)
    ).astype(np.float32)
    if run_kwargs:
        kernel.last_results = res
    return out


# revision 16
# speedup vs baseline: 1.0777x; 1.0777x over previous
"""MoE update-MLP Trainium2 kernel (8-core SPMD, sparse top-2 routing).

Problem: x (4,192,128,128); a per-pixel router picks top-2 of 8 experts; each
expert is a 3-layer 1x1-conv MLP (192->384 gelu ->384 gelu ->192); output is
the gate-weighted sum over experts.

Strategy (vs. the dense baseline that ran every expert on every pixel):
exploit the top-2 sparsity. The router is a tiny 8x8 per-pixel linear --
0.5% of the FLOPs -- so it is evaluated on the host, which then groups the
131072 (pixel, expert) pairs by expert, pads each expert's run to a multiple
of 512, and splits the resulting tile list evenly across the 8 cores
(perfect load balance even though the router is expert-skewed). Each 512-pair
tile is single-expert; the host streams that tile's weights (bf16, one
combined slab DMA per tile) alongside the gathered x columns, so the device
program is completely uniform and SPMD-identical -- all per-core/per-tile
variation lives in the input data.

Per 512-pair tile on each core (all matmuls bf16, PSUM fp32):
 - L1: K=192 split 96+96 (no zero pad), M=384 -> 6 matmuls, exact Gelu+bias
   on ACT -> h1 (bf16)
 - L2: K=384 (3), M=384 (3) -> 9 matmuls, Gelu+bias -> h2 (bf16)
 - L3: K=384 (3), M=192 (128+64) -> 6 matmuls; DVE copies PSUM into one
   bf16 output tile; single DMA out
The b3 bias, the gate weighting and the scatter back to pixel order all
happen on the host: out[pix] = g1*(y[q1]+b3[e1]) + g2*(y[q2]+b3[e2]); each
pixel has exactly one rank-1 and one rank-2 pair, so the combine is two
fancy-indexed gathers.

21 matmuls x 512 cols per tile, 33 tiles/core = ~1/4 of the dense baseline's
PE work.
"""

import numpy as np
from ml_dtypes import bfloat16

import concourse.bacc as bacc
import concourse.mybir as mybir
import concourse.tile as tile
from concourse.bass_utils import run_bass_kernel_spmd

F32 = mybir.dt.float32
BF16 = mybir.dt.bfloat16
AF = mybir.ActivationFunctionType
ALU = mybir.AluOpType

N_CORES = 8
B, IN_C, H, W = 4, 192, 128, 128
R_C, E, HID, OUT_C = 8, 8, 384, 192
P = B * H * W                 # 65536 pixels
TILE = 512                    # pixel-expert pairs per compute tile
NT = 33                       # tiles per core
NT_TOT = NT * N_CORES         # 264 tile capacity (needs >= ~257 on average)
KH = 96                       # L1 contraction split: 192 = 96 + 96

# weight-slab column layout (bf16): w1 | w2 | w3 | b1 | b2
_W1C = 0          # [:96, 384*k + 128*m] for k in 0..1
_W2C = 768        # [:,  768 + 384*k + 128*m] for k in 0..2
_W3C = 1920       # [:,  1920 + 192*k + 128*m] for k in 0..2
_B1C = 2496       # [:, 2496+m]
_B2C = 2499       # [:, 2499+m]
WC = 2504         # padded total columns

_nc_cache: dict = {}


def _build(compile: bool = True):
    """Build the (SPMD-identical) Bass program for one core."""
    nc = bacc.Bacc("TRN2", target_bir_lowering=False, debug=False)

    xg_in = nc.declare_dram_parameter("xg", [NT, KH, 2, TILE], BF16, isOutput=False)
    ws_in = nc.declare_dram_parameter("ws", [NT, 128, WC], BF16, isOutput=False)
    y_out = nc.declare_dram_parameter("y", [NT, 128, 2, TILE], BF16, isOutput=True)

    with tile.TileContext(nc) as tc:
        with (
            tc.tile_pool(name="xpool", bufs=3) as xpool,
            tc.tile_pool(name="wpool", bufs=3) as wpool,
            tc.tile_pool(name="hpool", bufs=8) as hpool,
            tc.tile_pool(name="ypool", bufs=3) as ypool,
            tc.tile_pool(name="psL1", bufs=3, space="PSUM") as psL1,
            tc.tile_pool(name="psL2", bufs=3, space="PSUM") as psL2,
            tc.tile_pool(name="psL3", bufs=2, space="PSUM") as psL3,
        ):
            for t in range(NT):
                x_sb = xpool.tile([KH, 2, TILE], BF16, tag="x")
                w_sb = wpool.tile([128, WC], BF16, tag="w")
                if t == 0:
                    # split the first weight slab so L1 can start on the
                    # (small, HWDGE-issued) w1 piece sooner
                    nc.sync.dma_start(w_sb[:, 0:768], ws_in[t, :, 0:768])
                    nc.sync.dma_start(x_sb[:], xg_in[t])
                    nc.gpsimd.dma_start(w_sb[:, 768:WC], ws_in[t, :, 768:WC])
                else:
                    nc.sync.dma_start(x_sb[:], xg_in[t])
                    nc.gpsimd.dma_start(w_sb[:], ws_in[t])

                h1 = []
                for m in range(3):
                    ps1 = psL1.tile([128, TILE], F32, tag="ps1")
                    for k in range(2):
                        c0 = _W1C + 384 * k + 128 * m
                        nc.tensor.matmul(
                            ps1[:],
                            w_sb[:KH, c0 : c0 + 128],
                            x_sb[:, k, :],
                            start=(k == 0),
                            stop=(k == 1),
                        )
                    h1_m = hpool.tile([128, TILE], BF16, tag="h1")
                    nc.scalar.activation(
                        h1_m[:], ps1[:], AF.Gelu,
                        bias=w_sb[:, _B1C + m : _B1C + m + 1],
                    )
                    h1.append(h1_m)

                h2 = []
                for m in range(3):
                    ps2 = psL2.tile([128, TILE], F32, tag="ps2")
                    for k in range(3):
                        c0 = _W2C + 384 * k + 128 * m
                        nc.tensor.matmul(
                            ps2[:],
                            w_sb[:, c0 : c0 + 128],
                            h1[k][:],
                            start=(k == 0),
                            stop=(k == 2),
                        )
                    h2_m = hpool.tile([128, TILE], BF16, tag="h2")
                    nc.scalar.activation(
                        h2_m[:], ps2[:], AF.Gelu,
                        bias=w_sb[:, _B2C + m : _B2C + m + 1],
                    )
                    h2.append(h2_m)

                y_sb = ypool.tile([128, 2, TILE], BF16, tag="y")
                for m, rows in ((0, 128), (1, OUT_C - 128)):
                    ps3 = psL3.tile([128, TILE], F32, tag="ps3")
                    for k in range(3):
                        c0 = _W3C + 192 * k + 128 * m
                        nc.tensor.matmul(
                            ps3[:rows],
                            w_sb[:, c0 : c0 + rows],
                            h2[k][:],
                            start=(k == 0),
                            stop=(k == 2),
                        )
                    nc.vector.tensor_copy(y_sb[:rows, m, :], ps3[:rows])
                    if t == NT - 1:
                        # last tile: ship each half as soon as it's copied so
                        # the 128-row transfer overlaps the 64-row L3 matmuls
                        nc.sync.dma_start(
                            y_out[t, :rows, m, :], y_sb[:rows, m, :]
                        )
                if t < NT - 1:
                    nc.sync.dma_start(y_out[t], y_sb[:])

    if compile:
        nc.compile()
    return nc


def _get_nc():
    if "nc" not in _nc_cache:
        _nc_cache["nc"] = _build()
    return _nc_cache["nc"]


def _route(router_input, router_W, router_b):
    """Host-side router: per-pixel top-2 experts and gates.

    Matches the reference: logits -> top-2 -> softmax over the two kept
    logits (2-way softmax == sigmoid of the difference).
    """
    f = np.float32
    r = np.asarray(router_input, f).reshape(B, R_C, H * W)
    Wr = np.asarray(router_W, f)
    br = np.asarray(router_b, f)
    lt = np.einsum("ec,bcp->bpe", Wr, r).reshape(P, E) + br[None, :]
    ord2 = np.argsort(-lt, axis=1, kind="stable")[:, :2]
    t1 = ord2[:, 0].astype(np.int32)
    t2 = ord2[:, 1].astype(np.int32)
    l12 = np.take_along_axis(lt, ord2, axis=1)
    g1 = 1.0 / (1.0 + np.exp(l12[:, 1] - l12[:, 0]))
    g2 = (1.0 - g1).astype(f)
    return t1, t2, g1.astype(f), g2


def _plan(t1, t2):
    """Group pairs by expert into 512-pair tiles; assign tiles to cores.

    Returns (cols, texp, q1, q2): cols[j] = source pixel of global pair
    column j (-1 = padding); texp[t] = expert of global tile t (-1 = empty);
    q1[p]/q2[p] = global pair column holding pixel p's rank-1/rank-2
    contribution (-1 if dropped on overflow -- cannot happen for this
    problem's 131072+padding <= 264*512, but handled for safety).
    """
    pixA = [np.flatnonzero(t1 == e) for e in range(E)]
    pixB = [np.flatnonzero(t2 == e) for e in range(E)]
    counts = np.array([len(pixA[e]) + len(pixB[e]) for e in range(E)])
    tiles_e = np.maximum(1, (counts + TILE - 1) // TILE)
    # overflow safety: drop pairs (rank-2 first) from the fullest experts
    while tiles_e.sum() > NT_TOT:
        e = int(np.argmax(counts - (tiles_e - 1) * TILE))
        drop = int(counts[e] - (tiles_e[e] - 1) * TILE)
        nB = len(pixB[e])
        db = min(drop, nB)
        pixB[e] = pixB[e][: nB - db]
        if db < drop:
            pixA[e] = pixA[e][: len(pixA[e]) - (drop - db)]
        counts[e] -= drop
        tiles_e[e] -= 1

    cols = np.full(NT_TOT * TILE, -1, np.int64)
    texp = np.full(NT_TOT, -1, np.int64)
    q1 = np.full(P, -1, np.int64)
    q2 = np.full(P, -1, np.int64)
    off = 0
    for e in range(E):
        nA, nB = len(pixA[e]), len(pixB[e])
        cols[off : off + nA] = pixA[e]
        cols[off + nA : off + nA + nB] = pixB[e]
        q1[pixA[e]] = off + np.arange(nA)
        q2[pixB[e]] = off + nA + np.arange(nB)
        t0 = off // TILE
        texp[t0 : t0 + tiles_e[e]] = e
        off += int(tiles_e[e]) * TILE
    return cols, texp, q1, q2


def _weight_slabs(W1, b1, W2, b2, W3):
    """Per-expert [128, WC] bf16 slabs in the device layout."""
    f = np.float32
    sl = np.zeros((E, 128, WC), f)
    w1t = np.transpose(np.asarray(W1, f), (0, 2, 1))  # [E, 192, HID]
    sl[:, :KH, _W1C : _W1C + 384] = w1t[:, :KH, :]
    sl[:, :KH, _W1C + 384 : _W1C + 768] = w1t[:, KH:, :]
    w2t = np.transpose(np.asarray(W2, f), (0, 2, 1))  # [E, 384, HID]
    for k in range(3):
        sl[:, :, _W2C + 384 * k : _W2C + 384 * (k + 1)] = w2t[
            :, 128 * k : 128 * (k + 1), :
        ]
    w3t = np.transpose(np.asarray(W3, f), (0, 2, 1))  # [E, 384, OUT_C]
    for k in range(3):
        sl[:, :, _W3C + 192 * k : _W3C + 192 * (k + 1)] = w3t[
            :, 128 * k : 128 * (k + 1), :
        ]
    b1a = np.asarray(b1, f).reshape(E, 3, 128)
    b2a = np.asarray(b2, f).reshape(E, 3, 128)
    for m in range(3):
        sl[:, :, _B1C + m] = b1a[:, m, :]
        sl[:, :, _B2C + m] = b2a[:, m, :]
    return sl.astype(bfloat16)


def make_in_maps(x, W1, b1, W2, b2, W3, cols, texp):
    f = np.float32
    wsl = _weight_slabs(W1, b1, W2, b2, W3)
    x16 = (
        np.asarray(x, f).transpose(1, 0, 2, 3).reshape(IN_C, P).astype(bfloat16)
    )  # [192, P]

    in_maps = []
    for c in range(N_CORES):
        colc = cols[c * NT * TILE : (c + 1) * NT * TILE]
        xc = np.zeros((IN_C, NT * TILE), bfloat16)
        valid = colc >= 0
        xc[:, valid] = x16[:, colc[valid]]
        xr = xc.reshape(IN_C, NT, TILE)
        xg = np.empty((NT, KH, 2, TILE), bfloat16)
        xg[:, :, 0, :] = xr[0:KH].transpose(1, 0, 2)
        xg[:, :, 1, :] = xr[KH : 2 * KH].transpose(1, 0, 2)
        tc_ = texp[c * NT : (c + 1) * NT]
        tcw = np.where(tc_ >= 0, tc_, 0)
        in_maps.append({"xg": xg, "ws": np.ascontiguousarray(wsl[tcw])})
    return in_maps


def kernel(x, router_input, router_W, router_b, W1, b1, W2, b2, W3, b3, **run_kwargs):
    nc = _get_nc()
    t1, t2, g1, g2 = _route(router_input, router_W, router_b)
    cols, texp, q1, q2 = _plan(t1, t2)
    in_maps = make_in_maps(x, W1, b1, W2, b2, W3, cols, texp)
    res = run_bass_kernel_spmd(nc, in_maps, list(range(N_CORES)), **run_kwargs)
    Y = np.concatenate(
        [np.asarray(res.results[c]["y"]) for c in range(N_CORES)], axis=0
    )  # [NT_TOT, 128, 2, TILE] bf16
    Yg = np.empty((OUT_C, NT_TOT * TILE), np.float32)
    Yg[0:128] = Y[:, :, 0, :].transpose(1, 0, 2).reshape(128, -1)
    Yg[128:OUT_C] = (
        Y[:, : OUT_C - 128, 1, :].transpose(1, 0, 2).reshape(OUT_C - 128, -1)
    )
    b3f = np.asarray(b3, np.float32)  # [E, OUT_C]
    if (q1 >= 0).all() and (q2 >= 0).all():
        outf = (Yg[:, q1] + b3f[t1].T) * g1[None, :] + (
            Yg[:, q2] + b3f[t2].T
        ) * g2[None, :]
    else:
        outf = np.zeros((OUT_C, P), np.float32)
        m1 = q1 >= 0
        m2 = q2 >= 0
        outf[:, m1] = (Yg[:, q1[m1]] + b3f[t1[m1]].T) * g1[m1][None, :]
        outf[:, m2] += (Yg[:, q2[m2]] + b3f[t2[m2]].T) * g2[m2][None, :]
    out = np.ascontiguousarray(
        outf.reshape(OUT_C, B, H, W).transpose(1, 0, 2, 3)
    ).astype(np.float32)
    if run_kwargs:
        kernel.last_results = res
    return out

